# revision 34
# baseline (speedup 1.0000x reference)
"""GroupLinear (MoE routing) Trainium2 kernel.

Problem: x [8,2048,1024] f32, group_by [8,2048] int32 in [0,8),
W [8, 1024*1024] f32 (row g -> (dout,din) weight), b [8,1024] f32.
out[b,s,:] = W[g].reshape(1024,1024) @ x[b,s,:] + b[g],  g = group_by[b,s].

Strategy: expert-parallel over 8 NeuronCores. Core g gets every token
routed to group g (host-side dispatch), its own weight (pre-transposed to
[din, dout]), and bias. Tokens beyond the C capacity finish on the host.
Host scatters the per-core results back to token order.

Schedule variants (see _emit_*):
  base   - stationary = X^T tile [128d,128t], moving = W [128d,512o],
           y [C, DOUT] f32 out.  ~71.4us/iter (the 70767ns baseline).
  base16 - base but y emitted as f16 (halves output DMA). ~68us.
  wstat  - stationary = W chunk [128d,128o] (reused across 4 moving
           token chunks via k-outer/tc-inner loop), moving = X
           [128d,512t], x fully SBUF-resident (double-buffered across
           reps), y [DOUT, C] f16 out (host transposes back).
  wstatd - wstat + post-Tile removal of redundant Ldweights (the 3
           repeats per stationary reuse group).
  *_sp   - output DMA on the SP HWDGE ring (nc.sync) instead of GpSimd
           SWDGE, keeping the ACT ring dedicated to x loads.

Best: wstatd_sp at ~60.7-61.2us/iter same-session vs base 71.4
(256 matmuls of N=512 f16: ~55.3us streaming floor at 2.4GHz; the
remainder is psum-release waits, rep edges, and device power state).
Findings from microbenchmarks (_emit_mraw): back-to-back N=512 f16
matmul streams run at the same rate regardless of stationary switching
or accumulation-group length; under sustained 8-core load the PE
downclocks ~2.4->2.0GHz, so measured per-iter time depends on the
measurement duty cycle.
"""

import numpy as np
from contextlib import ExitStack

import concourse.bass as bass
import concourse.mybir as mybir
import concourse.tile as tile
from concourse import bacc
from concourse.bass_utils import run_bass_kernel_spmd

B, S, DIN, DOUT, G = 8, 2048, 1024, 1024, 8
P = 128
KC = DIN // P     # 8 contraction chunks
OH = DOUT // 512  # 2 moving halves in base schedule

C_DEFAULT = 2048          # per-core token capacity (16 * 128)
DT_DEFAULT = "f16"        # matmul operand dtype: f32r | f16 | bf16
V_DEFAULT = "wstatd6b3_sp"  # schedule variant

_cache = {}


def _emit_base(ctx, tc, y, xt, wt, bias, C, mdt, odt, reps=1):
    nc = tc.nc
    f32 = mybir.dt.float32
    TB = C // P

    singles = ctx.enter_context(tc.tile_pool(name="singles", bufs=1))
    xpool = ctx.enter_context(tc.tile_pool(name="xpool", bufs=8))
    opool = ctx.enter_context(tc.tile_pool(name="opool", bufs=4))
    psum = ctx.enter_context(tc.tile_pool(name="psum", bufs=8, space="PSUM"))

    xt_r = xt.rearrange("(k p) t -> p k t", p=P)
    PH0 = 4  # t-blocks covered by the k-outer warmup phase

    def load_xt(tb):
        xt_tile = xpool.tile([P, KC, P], mdt, name="xt_tile", tag="xt_tile")
        nc.scalar.dma_start(out=xt_tile, in_=xt_r[:, :, tb * P:(tb + 1) * P])
        return xt_tile

    prefetched = {tb: load_xt(tb) for tb in range(PH0)}

    wt_sb = singles.tile([P, KC, DOUT], mdt)
    wt_r = wt.rearrange("(k p) o -> p k o", p=P)
    for k in range(KC):
        nc.sync.dma_start(out=wt_sb[:, k, :], in_=wt_r[:, k, :])
    bias_sb = singles.tile([P, DOUT], f32)
    nc.sync.dma_start(out=bias_sb, in_=bias)

    def emit_out(ps, tb, oh):
        ot = opool.tile([P, 512], odt, name="ot", tag="ot")
        nc.vector.tensor_add(out=ot, in0=ps, in1=bias_sb[:, oh * 512:(oh + 1) * 512])
        nc.gpsimd.dma_start(out=y[tb * P:(tb + 1) * P, oh * 512:(oh + 1) * 512], in_=ot)

    def mm(ps, xt_tile, k, oh):
        nc.tensor.matmul(
            ps,
            lhsT=xt_tile[:, k, :],
            rhs=wt_sb[:, k, oh * 512:(oh + 1) * 512],
            start=(k == 0),
            stop=(k == KC - 1),
        )

    for _rep in range(reps):
        if _rep == 0:
            tiles0 = [prefetched.pop(tb) for tb in range(PH0)]
            ps0 = [psum.tile([P, 512], f32, name="ps", tag="ps")
                   for _ in range(PH0 * OH)]
            for k in range(KC):
                for i in range(PH0 * OH):
                    tb, oh = divmod(i, OH)
                    mm(ps0[i], tiles0[tb], k, oh)
            for i in range(PH0 * OH):
                tb, oh = divmod(i, OH)
                emit_out(ps0[i], tb, oh)
        start_tb = PH0 if _rep == 0 else 0
        for tb in range(start_tb, TB):
            xt_tile = load_xt(tb)
            for oh in range(OH):
                ps = psum.tile([P, 512], f32, name="ps", tag="ps")
                for k in range(KC):
                    mm(ps, xt_tile, k, oh)
                emit_out(ps, tb, oh)


def _emit_wstat(ctx, tc, y, xt, wt, bias, C, mdt, odt, reps=1, mode="full",
                out_eng="gpsimd", xbig=False, xbufs=2):
    """W-stationary: for each (od, k) the stationary W[128d,128o] chunk is
    reused across all TC moving token chunks. x is fully SBUF-resident
    (double-buffered across reps). y is [DOUT, C] (transposed).

    mode: "full" | "mmonly" (no eviction/out-DMA; diagnostic)
        | "nodma" (eviction but no DMAs per rep; diagnostic)."""
    nc = tc.nc
    f32 = mybir.dt.float32
    TC = C // 512
    OD = DOUT // P  # 8 dout chunks

    singles = ctx.enter_context(tc.tile_pool(name="singles", bufs=1))
    xpool = ctx.enter_context(
        tc.tile_pool(name="xpool", bufs=xbufs if mode != "nodma" else 1))
    opool = ctx.enter_context(
        tc.tile_pool(name="opool", bufs=2 if mode == "bigout" else 6))
    psum = ctx.enter_context(tc.tile_pool(name="psum", bufs=8, space="PSUM"))

    ws = singles.tile([P, KC, DOUT], mdt)
    wt_r = wt.rearrange("(k p) o -> p k o", p=P)
    for k in range(KC):
        nc.sync.dma_start(out=ws[:, k, :], in_=wt_r[:, k, :])
    bias_sb = singles.tile([P, OD], f32)
    nc.sync.dma_start(out=bias_sb, in_=bias)

    xt_r = xt.rearrange("(k p) t -> p k t", p=P)

    y_r = (y.rearrange("(o p) t -> p o t", p=P)
           if mode in ("bigout", "coalesce2") else None)

    xs = None
    last_ps = None
    for _rep in range(reps):
        if mode == "bigout":
            otw = opool.tile([P, OD, C], odt, name="otb", tag="otb")
        if mode == "nodma":
            if xs is None:
                xs = xpool.tile([P, KC, C], mdt, name="xs", tag="xs")
                for k in range(KC):
                    nc.scalar.dma_start(out=xs[:, k, :], in_=xt_r[:, k, :])
        else:
            xs = xpool.tile([P, KC, C], mdt, name="xs", tag="xs")
            if xbig:
                nc.scalar.dma_start(out=xs, in_=xt_r)
            else:
                for k in range(KC):
                    nc.scalar.dma_start(out=xs[:, k, :], in_=xt_r[:, k, :])
        for od in range(OD):
            pss = [psum.tile([P, 512], f32, name="ps", tag="ps")
                   for _ in range(TC)]
            if mode == "kinner":
                for tci in range(TC):
                    for k in range(KC):
                        nc.tensor.matmul(
                            pss[tci],
                            lhsT=ws[:, k, od * P:(od + 1) * P],
                            rhs=xs[:, k, tci * 512:(tci + 1) * 512],
                            start=(k == 0),
                            stop=(k == KC - 1),
                        )
            else:
                for k in range(KC):
                    for tci in range(TC):
                        nc.tensor.matmul(
                            pss[tci],
                            lhsT=ws[:, k, od * P:(od + 1) * P],
                            rhs=xs[:, k, tci * 512:(tci + 1) * 512],
                            start=(k == 0),
                            stop=(k == KC - 1),
                        )
            last_ps = pss[-1]
            if mode == "mmonly":
                continue
            if mode == "coalesce":
                # fine-grained per-bank evictions into one wide staging
                # tile, then a single 4KB-per-partition output DMA per od
                ot = opool.tile([P, C], odt, name="otw", tag="otw")
                for tci in range(TC):
                    nc.vector.tensor_scalar_add(
                        out=ot[:, tci * 512:(tci + 1) * 512],
                        in0=pss[tci], scalar1=bias_sb[:, od:od + 1])
                getattr(nc, out_eng).dma_start(
                    out=y[od * P:(od + 1) * P, :], in_=ot)
                continue
            if mode == "coalesce2":
                # stage TWO ods, one 8KB-per-partition DMA per od pair
                if od % 2 == 0:
                    ot2 = opool.tile([P, 2, C], odt, name="ot2", tag="ot2")
                for tci in range(TC):
                    nc.vector.tensor_scalar_add(
                        out=ot2[:, od % 2, tci * 512:(tci + 1) * 512],
                        in0=pss[tci], scalar1=bias_sb[:, od:od + 1])
                if od % 2 == 1:
                    getattr(nc, out_eng).dma_start(
                        out=y_r[:, od - 1:od + 1, :], in_=ot2)
                continue
            if mode == "bigout":
                # stage the whole rep's output; one giant DMA at rep end
                for tci in range(TC):
                    nc.vector.tensor_scalar_add(
                        out=otw[:, od, tci * 512:(tci + 1) * 512],
                        in0=pss[tci], scalar1=bias_sb[:, od:od + 1])
                if od == OD - 1:
                    getattr(nc, out_eng).dma_start(out=y_r, in_=otw)
                continue
            for tci in range(TC):
                ot = opool.tile([P, 512], odt, name="ot", tag="ot")
                nc.vector.tensor_scalar_add(
                    out=ot, in0=pss[tci], scalar1=bias_sb[:, od:od + 1])
                if mode != "nodma":
                    getattr(nc, out_eng).dma_start(
                        out=y[od * P:(od + 1) * P,
                              tci * 512:(tci + 1) * 512],
                        in_=ot)
    if mode in ("mmonly", "nodma"):
        ot = opool.tile([P, 512], odt, name="otf", tag="ot")
        nc.vector.tensor_scalar_add(out=ot, in0=last_ps,
                                    scalar1=bias_sb[:, 0:1])
        nc.gpsimd.dma_start(out=y[0:P, 0:512], in_=ot)


def _emit_wstat2(ctx, tc, y, xt, wt, bias, C, mdt, odt, reps=1,
                 out_eng="sync"):
    """Like _emit_wstat but each od accumulates into ONE [128, C] PSUM tile
    spanning TC banks (each matmul still targets a single bank slice).
    One eviction + one output DMA per od."""
    nc = tc.nc
    f32 = mybir.dt.float32
    TC = C // 512
    OD = DOUT // P

    singles = ctx.enter_context(tc.tile_pool(name="singles", bufs=1))
    xpool = ctx.enter_context(tc.tile_pool(name="xpool", bufs=2))
    opool = ctx.enter_context(tc.tile_pool(name="opool", bufs=3))
    psum = ctx.enter_context(tc.tile_pool(name="psum", bufs=2, space="PSUM"))

    ws = singles.tile([P, KC, DOUT], mdt)
    wt_r = wt.rearrange("(k p) o -> p k o", p=P)
    for k in range(KC):
        nc.sync.dma_start(out=ws[:, k, :], in_=wt_r[:, k, :])
    bias_sb = singles.tile([P, OD], f32)
    nc.sync.dma_start(out=bias_sb, in_=bias)

    xt_r = xt.rearrange("(k p) t -> p k t", p=P)

    for _rep in range(reps):
        xs = xpool.tile([P, KC, C], mdt, name="xs", tag="xs")
        for k in range(KC):
            nc.scalar.dma_start(out=xs[:, k, :], in_=xt_r[:, k, :])
        for od in range(OD):
            ps = psum.tile([P, C], f32, name="ps", tag="ps")
            for k in range(KC):
                for tci in range(TC):
                    nc.tensor.matmul(
                        ps[:, tci * 512:(tci + 1) * 512],
                        lhsT=ws[:, k, od * P:(od + 1) * P],
                        rhs=xs[:, k, tci * 512:(tci + 1) * 512],
                        start=(k == 0),
                        stop=(k == KC - 1),
                    )
            ot = opool.tile([P, C], odt, name="ot", tag="ot")
            nc.vector.tensor_scalar_add(
                out=ot, in0=ps, scalar1=bias_sb[:, od:od + 1])
            getattr(nc, out_eng).dma_start(
                out=y[od * P:(od + 1) * P, :], in_=ot)


def _emit_mraw(ctx, tc, y, xt, wt, bias, C, mdt, odt, reps=1,
               switch=True, grp=8):
    """PE microbench: 256 matmuls/rep of N=512. switch: rotate stationary
    every MM vs keep it fixed. grp: accumulation group length (1 = every MM
    start+stop, 8 = 8-MM groups). x/W resident; no evictions, no per-rep
    DMA. Output is garbage except a final dummy tile."""
    nc = tc.nc
    f32 = mybir.dt.float32

    singles = ctx.enter_context(tc.tile_pool(name="singles", bufs=1))
    opool = ctx.enter_context(tc.tile_pool(name="opool", bufs=2))
    psum = ctx.enter_context(tc.tile_pool(name="psum", bufs=8, space="PSUM"))

    ws = singles.tile([P, KC, DOUT], mdt)
    wt_r = wt.rearrange("(k p) o -> p k o", p=P)
    for k in range(KC):
        nc.sync.dma_start(out=ws[:, k, :], in_=wt_r[:, k, :])
    bias_sb = singles.tile([P, DOUT // P], f32)
    nc.sync.dma_start(out=bias_sb, in_=bias)
    xs = singles.tile([P, KC, C], mdt)
    xt_r = xt.rearrange("(k p) t -> p k t", p=P)
    for k in range(KC):
        nc.scalar.dma_start(out=xs[:, k, :], in_=xt_r[:, k, :])

    TC = C // 512
    n_mm = 32 * KC  # 256 per rep
    ps = None
    for _rep in range(reps):
        for i in range(n_mm):
            k = i % KC
            if i % grp == 0:
                ps = psum.tile([P, 512], f32, name="ps", tag="ps")
            nc.tensor.matmul(
                ps,
                lhsT=ws[:, k if switch else 0, 0:P],
                rhs=xs[:, k, (i // KC % TC) * 512:(i // KC % TC + 1) * 512],
                start=(i % grp == 0),
                stop=(i % grp == grp - 1),
                skip_group_check=True,
            )
    ot = opool.tile([P, 512], odt, name="ot", tag="ot")
    nc.vector.tensor_scalar_add(out=ot, in0=ps, scalar1=bias_sb[:, 0:1])
    nc.gpsimd.dma_start(out=y[0:P, 0:512], in_=ot)


def _dedup_ldweights(nc):
    """Remove Ldweights whose weights AP is identical to the previously
    loaded one and which carry no sync info. Must run after TileContext
    exit (sems assigned) and before nc.compile()."""
    n_removed = 0
    for blk in nc.m.functions[0].blocks:
        insts = blk.instructions
        cur_sig = None
        i = 0
        while i < len(insts):
            inst = insts[i]
            if isinstance(inst, mybir.InstLdweights):
                sig = str(inst.ins[0])
                si = inst.sync_info
                has_sync = si is not None and (
                    len(si.on_wait) > 0 or len(si.on_update) > 0)
                if sig == cur_sig and not has_sync:
                    del insts[i]
                    n_removed += 1
                    continue
                cur_sig = sig
            elif (inst.engine == mybir.EngineType.PE
                  and not isinstance(inst, mybir.InstMatmult)):
                cur_sig = None  # other PE inst invalidates loaded weights
            i += 1
    return n_removed


def _build(reps=1, C=C_DEFAULT, dt=DT_DEFAULT, variant=V_DEFAULT):
    key = (reps, C, dt, variant)
    if key in _cache:
        return _cache[key]
    nc = bacc.Bacc("TRN2", target_bir_lowering=False, debug=False,
                   enable_asserts=False, num_devices=G)
    f32 = mybir.dt.float32
    f16 = mybir.dt.float16
    mdt = {"f32r": mybir.dt.float32r, "f16": mybir.dt.float16,
           "bf16": mybir.dt.bfloat16}[dt]
    in_dt = mdt if dt != "f32r" else mybir.dt.float32r
    odt = f32 if variant == "base" else f16

    xt = nc.dram_tensor("xt", [DIN, C], in_dt, kind="ExternalInput").ap()
    wt = nc.dram_tensor("wt", [DIN, DOUT], in_dt, kind="ExternalInput").ap()
    if variant in ("base", "base16"):
        bias = nc.dram_tensor("bias", [P, DOUT], f32, kind="ExternalInput").ap()
        y = nc.dram_tensor("y", [C, DOUT], odt, kind="ExternalOutput").ap()
        emit = _emit_base
        kwargs = {}
    else:
        bias = nc.dram_tensor("bias", [P, DOUT // P], f32,
                              kind="ExternalInput").ap()
        y = nc.dram_tensor("y", [DOUT, C], odt, kind="ExternalOutput").ap()
        if variant.startswith("mraw"):
            emit = _emit_mraw
            kwargs = {
                "mraw_same_g1": {"switch": False, "grp": 1},
                "mraw_sw_g1": {"switch": True, "grp": 1},
                "mraw_same_g8": {"switch": False, "grp": 8},
                "mraw_sw_g8": {"switch": True, "grp": 8},
            }[variant]
        elif variant.startswith("wstat2") or variant.startswith("wstatd2"):
            emit = _emit_wstat2
            kwargs = {}
        else:
            emit = _emit_wstat
            kwargs = {"mode": {"mmonly": "mmonly", "nodma": "nodma",
                               "wstat_ki_sp": "kinner",
                               "wstat3_sp": "coalesce",
                               "wstatd3_sp": "coalesce",
                               "wstat4_sp": "bigout",
                               "wstatd4_sp": "bigout",
                               "wstatd5_sp": "coalesce",
                               "wstatd6_sp": "coalesce2",
                               "wstatd6b3_sp": "coalesce2"}.get(variant, "full")}
            if variant.endswith("_sp"):
                kwargs["out_eng"] = "sync"
            if variant == "wstatd5_sp":
                kwargs["xbig"] = True
            if variant == "wstatd6b3_sp":
                kwargs["xbufs"] = 3
    with tile.TileContext(nc) as tc, ExitStack() as ctx:
        emit(ctx, tc, y, xt, wt, bias, C, mdt, odt, reps=reps, **kwargs)
    if variant.startswith("wstatd"):
        n = _dedup_ldweights(nc)
        assert n > 0, "expected to remove redundant Ldweights"
    nc.compile()
    _cache[key] = nc
    return nc


def _prep_inputs(x, group_by, W, b, C=C_DEFAULT, dt=DT_DEFAULT,
                 variant=V_DEFAULT):
    import ml_dtypes
    np_dt = np.float32 if dt == "f32r" else (
        np.float16 if dt == "f16" else ml_dtypes.bfloat16)
    x_flat = np.ascontiguousarray(np.asarray(x, dtype=np.float32)).reshape(B * S, DIN)
    gb = np.asarray(group_by).reshape(B * S)
    W = np.asarray(W, dtype=np.float32)
    b = np.asarray(b, dtype=np.float32)

    idxs, in_maps = [], []
    for g in range(G):
        idx = np.nonzero(gb == g)[0]
        n = min(len(idx), C)
        xt = np.zeros((DIN, C), dtype=np_dt)
        xt[:, :n] = x_flat[idx[:n]].T.astype(np_dt)
        wtg = np.ascontiguousarray(W[g].reshape(DOUT, DIN).T.astype(np_dt))
        if variant in ("base", "base16"):
            bias = np.ascontiguousarray(np.broadcast_to(b[g], (P, DOUT)))
        else:
            bias = np.ascontiguousarray(b[g].reshape(DOUT // P, P).T)
        in_maps.append({"xt": xt, "wt": wtg, "bias": bias})
        idxs.append(idx)
    return x_flat, idxs, in_maps, W, b


def _scatter(results, x_flat, idxs, W, b, C=C_DEFAULT, variant=V_DEFAULT):
    out_flat = np.empty((B * S, DOUT), dtype=np.float32)
    for g in range(G):
        idx = idxs[g]
        n = min(len(idx), C)
        yg = results[g]["y"]
        if variant in ("base", "base16"):
            out_flat[idx[:n]] = yg[:n].astype(np.float32)
        else:
            out_flat[idx[:n]] = yg[:, :n].T.astype(np.float32)
        if len(idx) > C:  # capacity spill: finish the stragglers on host
            extra = idx[C:]
            out_flat[extra] = x_flat[extra] @ W[g].reshape(DOUT, DIN).T + b[g]
    return out_flat.reshape(B, S, DOUT)


def kernel(x, group_by, W, b):
    nc = _build()
    x_flat, idxs, in_maps, W, b = _prep_inputs(x, group_by, W, b)
    res = run_bass_kernel_spmd(nc, in_maps, list(range(G)))
    return _scatter(res.results, x_flat, idxs, W, b)
